# revision 10
# baseline (speedup 1.0000x reference)
"""Bass/Trainium2 kernel for nn_BipartiteSAGELayer (bipartite GraphSAGE layer).

Strategy (8 NeuronCores, SPMD, no collectives):
  - Shard DESTINATION nodes across cores: core c owns jobs [c*12500, (c+1)*12500)
    and machines [c*1250, (c+1)*1250).
  - Host prep ("sharding"): sort/bucket edges per (core, dest-window-of-128
    [, source-bucket-of-32768]) with padding so every core runs an identical
    static program; cast feature tables to bf16; pre-transpose per-core node
    features.
  - Device per dest-window: dma_gather source rows (bf16, 256B) from the DRAM
    table; build one-hot [edge, dest] tiles on DVE (is_equal vs an iota row);
    TensorE: agg[d,f] += onehot.T @ gathered, deg[d] += onehot.T @ ones;
    scale rows by 1/clip(deg,1) (ACT copy with per-partition scale), PE
    transpose -> aggT; linear new = [h | agg] @ W.T via hT/aggT stationaries;
    relu+LN (batched moment math) -> output rows.
"""

import numpy as np
import ml_dtypes

N_JOBS = 100000
N_MACH = 10000
N_EDGES = 1600000
D = 128
OUT = 128
LN_EPS = 1e-5
N_CORES = 8
JPC = N_JOBS // N_CORES       # 12500
MPC = N_MACH // N_CORES       # 1250
BUCKET = 32768                # int16-safe source bucket for the job table
GB = 64                       # max chunks (of 128 edges) per dma_gather block

bf16 = ml_dtypes.bfloat16


# ---------------------------------------------------------------- host prep

def _pad128(x):
    return (int(x) + 127) // 128 * 128


def _prep_pass(dst, src, n_dest_pc, n_buckets, bucket_size):
    """Shard edges by (core, dest-window [,src-bucket]) with a static layout.

    Returns (groups, T, src_l, dst_l):
      groups: per window, list of (bucket, n_chunks) with n_chunks >= 0;
              layout is window-major, bucket-sub-major, each group padded
              to a multiple of 128 slots. Identical across cores.
      T: total slots per core.
      src_l: [n_cores, T] int16 bucket-local source indices (pads = 0)
      dst_l: [n_cores, T] float32 window-local dest indices (pads = -1)
    """
    dst = dst.astype(np.int64)
    src = src.astype(np.int64)
    n_win = _pad128(n_dest_pc) // 128
    core = dst // n_dest_pc
    dloc = dst % n_dest_pc
    win = dloc // 128
    buck = src // bucket_size if n_buckets > 1 else np.zeros_like(src)
    nk = N_CORES * n_win * n_buckets
    key = (core * n_win + win) * n_buckets + buck
    cnt = np.bincount(key, minlength=nk).reshape(N_CORES, n_win, n_buckets)
    grp = cnt.max(axis=0)                      # [n_win, n_buckets]
    grp_pad = (grp + 127) // 128 * 128
    for w in range(n_win):                     # ensure >=1 chunk per window
        if grp_pad[w].sum() == 0:
            grp_pad[w, 0] = 128
    # flat offsets (same for every core)
    off = np.zeros((n_win, n_buckets), np.int64)
    t = 0
    groups = []
    for w in range(n_win):
        gl = []
        for b in range(n_buckets):
            off[w, b] = t
            t += grp_pad[w, b]
            if grp_pad[w, b]:
                gl.append((b, int(grp_pad[w, b]) // 128))
        groups.append(gl)
    T = int(t)

    # slot of each edge: off[win, buck] + rank within its (core,win,buck) group
    order = np.argsort(key, kind="stable")
    sk = key[order]
    first = np.zeros(nk, np.int64)
    first[1:] = np.cumsum(np.bincount(key, minlength=nk))[:-1]
    rank = np.arange(len(dst)) - first[sk]
    e_core = core[order]
    e_win = win[order]
    e_buck = buck[order]
    slot = off[e_win, e_buck] + rank

    src_l = np.zeros((N_CORES, T), np.int16)
    dst_l = np.full((N_CORES, T), -1.0, np.float32)
    src_l[e_core, slot] = (src[order] - e_buck * bucket_size).astype(np.int16)
    dst_l[e_core, slot] = (dloc[order] - e_win * 128).astype(np.float32)
    return groups, T, src_l, dst_l


def _wrap_idx(src_l):
    """[T] int16 -> [128, T//16] gather-index layout (16-wrapped, tiled x8)."""
    T = src_l.shape[-1]
    w16 = src_l.reshape(-1, T // 16, 16).transpose(0, 2, 1)  # [cores, 16, T/16]
    return np.ascontiguousarray(np.tile(w16, (1, 8, 1)))     # [cores, 128, T/16]


def _wrap_dst(dst_l):
    """[T] f32 -> [128, T//128] bf16 per-chunk dest columns."""
    T = dst_l.shape[-1]
    d = dst_l.reshape(-1, T // 128, 128).transpose(0, 2, 1)
    return np.ascontiguousarray(d).astype(bf16)


# ---------------------------------------------------------------- device build

def _build(cfg):
    import concourse.bacc as bacc
    import concourse.tile as tile
    import concourse.mybir as mybir
    from concourse import library_config

    f32 = mybir.dt.float32
    b16 = mybir.dt.bfloat16
    i16 = mybir.dt.int16
    Alu = mybir.AluOpType
    Act = mybir.ActivationFunctionType

    nc = bacc.Bacc("TRN2", target_bir_lowering=False)

    mach_tab = nc.dram_tensor("mach_tab", [cfg["n_mach_tab"], D], b16, kind="ExternalInput")
    job_tab = nc.dram_tensor("job_tab", [cfg["n_job_tab"], D], b16, kind="ExternalInput")
    jobT = nc.dram_tensor("jobT", [128, cfg["nj_pad"]], b16, kind="ExternalInput")
    machT = nc.dram_tensor("machT", [128, cfg["nm_pad"]], b16, kind="ExternalInput")
    idx1_d = nc.dram_tensor("idx1", [128, cfg["T1"] // 16], i16, kind="ExternalInput")
    dst1_d = nc.dram_tensor("dst1", [128, cfg["T1"] // 128], b16, kind="ExternalInput")
    idx2_d = nc.dram_tensor("idx2", [128, cfg["T2"] // 16], i16, kind="ExternalInput")
    dst2_d = nc.dram_tensor("dst2", [128, cfg["T2"] // 128], b16, kind="ExternalInput")
    wjt1_d = nc.dram_tensor("wjt1", [128, 128], b16, kind="ExternalInput")
    wjt2_d = nc.dram_tensor("wjt2", [128, 128], b16, kind="ExternalInput")
    wmt1_d = nc.dram_tensor("wmt1", [128, 128], b16, kind="ExternalInput")
    wmt2_d = nc.dram_tensor("wmt2", [128, 128], b16, kind="ExternalInput")
    bj_d = nc.dram_tensor("bj", [1, 128], b16, kind="ExternalInput")
    bm_d = nc.dram_tensor("bm", [1, 128], b16, kind="ExternalInput")
    iota_d = nc.dram_tensor("iota", [128, 128], b16, kind="ExternalInput")
    ident_d = nc.dram_tensor("ident", [128, 128], b16, kind="ExternalInput")
    lnj_d = nc.dram_tensor("lnj", [128, 2, 128], f32, kind="ExternalInput")
    lnm_d = nc.dram_tensor("lnm", [128, 2, 128], f32, kind="ExternalInput")
    outj_d = nc.dram_tensor("outj", [cfg["nj_pad"], 128], f32, kind="ExternalOutput")
    outm_d = nc.dram_tensor("outm", [cfg["nm_pad"], 128], f32, kind="ExternalOutput")

    NWJ = cfg["nj_pad"] // 128
    NWM = cfg["nm_pad"] // 128

    with tile.TileContext(nc) as tc:
        with (
            tc.tile_pool(name="consts", bufs=1) as consts,
            tc.tile_pool(name="big", bufs=1) as big,
            tc.tile_pool(name="gpool", bufs=2) as gpool,
            tc.tile_pool(name="work", bufs=3) as work,
            tc.tile_pool(name="small", bufs=3) as small,
            tc.tile_pool(name="psum", bufs=2, space="PSUM") as psum,
        ):
            nc.gpsimd.load_library(library_config.mlp)

            iota_t = consts.tile([128, 128], b16)
            ident_t = consts.tile([128, 128], b16)
            onec_t = consts.tile([128, 1], b16)
            ones1_t = consts.tile([1, 128], b16)
            wjt1_t = consts.tile([128, 128], b16)
            wjt2_t = consts.tile([128, 128], b16)
            wmt1_t = consts.tile([128, 128], b16)
            wmt2_t = consts.tile([128, 128], b16)
            bj_t = consts.tile([1, 128], b16)
            bm_t = consts.tile([1, 128], b16)
            nc.sync.dma_start(iota_t[:], iota_d[:])
            nc.sync.dma_start(ident_t[:], ident_d[:])
            nc.sync.dma_start(wjt1_t[:], wjt1_d[:])
            nc.sync.dma_start(wjt2_t[:], wjt2_d[:])
            nc.sync.dma_start(wmt1_t[:], wmt1_d[:])
            nc.sync.dma_start(wmt2_t[:], wmt2_d[:])
            nc.sync.dma_start(bj_t[:], bj_d[:])
            nc.sync.dma_start(bm_t[:], bm_d[:])
            nc.vector.memset(onec_t[:], 1.0)
            nc.vector.memset(ones1_t[:], 1.0)
            eps_t = consts.tile([128, 1], f32)
            nc.vector.memset(eps_t[:], float(LN_EPS))
            lnj_t = lnm_t = None
            if cfg["affine_j"]:
                lnj_t = consts.tile([128, 2, 128], f32)
                nc.sync.dma_start(lnj_t[:], lnj_d[:])
            if cfg["affine_m"]:
                lnm_t = consts.tile([128, 2, 128], f32)
                nc.sync.dma_start(lnm_t[:], lnm_d[:])

            def emit_pass(n_win, groups, idx_d_, dst_d_, hT_d, hT_cols, table_d,
                          table_rows, bucket_size, WT1, WT2, bias_t, use_bias,
                          ln_t, out_d):
                idx_t = big.tile([128, max(cfg["T1"], cfg["T2"]) // 16], i16, tag="idx")
                dst_t = big.tile([128, max(cfg["T1"], cfg["T2"]) // 128], b16, tag="dst")
                hT_t = big.tile([128, max(cfg["nj_pad"], cfg["nm_pad"])], b16, tag="hT")
                Tn = idx_d_.shape[1]
                nc.sync.dma_start(idx_t[:, :Tn], idx_d_[:])
                nc.sync.dma_start(dst_t[:, :Tn // 8], dst_d_[:])
                nc.sync.dma_start(hT_t[:, :hT_cols], hT_d[:])

                slab = big.tile([128, max(NWJ, NWM), 128], f32, tag="slab")
                sums_t = big.tile([128, max(NWJ, NWM)], f32, tag="sums")
                sumsq_t = big.tile([128, max(NWJ, NWM)], f32, tag="sumsq")

                chunk_off = 0
                for w in range(n_win):
                    agg_p = psum.tile([128, 128], f32, tag="agg")
                    deg_p = psum.tile([128, 1], f32, tag="deg")
                    wch = sum(nch for _, nch in groups[w])
                    ci = 0
                    for (b, nch) in groups[w]:
                        lo = b * bucket_size
                        hi = min((b + 1) * bucket_size, table_rows)
                        tab_ap = table_d[lo:hi, :]
                        for blk in range(0, nch, GB):
                            nb = min(GB, nch - blk)
                            co = chunk_off + ci
                            g_t = gpool.tile([128, GB, 128], b16, tag="gath")
                            nc.gpsimd.dma_gather(
                                g_t[:, :nb, :], tab_ap,
                                idx_t[:, co * 8:(co + nb) * 8],
                                nb * 128, nb * 128, 128, single_packet=False)
                            oh_t = gpool.tile([128, GB, 128], b16, tag="oh")
                            nc.vector.tensor_tensor(
                                oh_t[:, :nb, :],
                                dst_t[:, co:co + nb].unsqueeze(2).broadcast_to([128, nb, 128]),
                                iota_t[:].unsqueeze(1).broadcast_to([128, nb, 128]),
                                Alu.is_equal)
                            for c in range(nb):
                                st = (ci == 0)
                                sp = (ci == wch - 1)
                                nc.tensor.matmul(agg_p[:], oh_t[:, c, :], g_t[:, c, :],
                                                 start=st, stop=sp)
                                nc.tensor.matmul(deg_p[:], oh_t[:, c, :], onec_t[:],
                                                 start=st, stop=sp)
                                ci += 1
                    chunk_off += wch

                    degc_t = small.tile([128, 1], f32, tag="degc")
                    nc.vector.tensor_scalar_max(degc_t[:], deg_p[:], 1.0)
                    recip_t = small.tile([128, 1], f32, tag="recip")
                    nc.vector.reciprocal(recip_t[:], degc_t[:])
                    mean_s = work.tile([128, 128], b16, tag="means")
                    nc.scalar.activation(mean_s[:], agg_p[:], Act.Copy,
                                         bias=0.0, scale=recip_t[:])
                    meanT_p = psum.tile([128, 128], b16, tag="mT")
                    nc.tensor.transpose(meanT_p[:], mean_s[:], ident_t[:])
                    aggT_s = work.tile([128, 128], b16, tag="aggT")
                    nc.scalar.activation(aggT_s[:], meanT_p[:], Act.Copy)

                    pre_p = psum.tile([128, 128], f32, tag="lin")
                    nc.tensor.matmul(pre_p[:], hT_t[:, w * 128:(w + 1) * 128], WT1[:],
                                     start=True, stop=False)
                    nc.tensor.matmul(pre_p[:], aggT_s[:], WT2[:],
                                     start=False, stop=not use_bias)
                    if use_bias:
                        nc.tensor.matmul(pre_p[:], ones1_t[:], bias_t[:],
                                         start=False, stop=True)
                    nc.scalar.activation(slab[:, w, :], pre_p[:], Act.Relu,
                                         accum_out=sums_t[:, w:w + 1])
                    sq_t = work.tile([128, 128], f32, tag="sqscr")
                    nc.scalar.activation(sq_t[:], slab[:, w, :], Act.Square,
                                         accum_out=sumsq_t[:, w:w + 1])

                # batched LN moments
                negmu_t = big.tile([128, max(NWJ, NWM)], f32, tag="negmu")
                rstd_t = big.tile([128, max(NWJ, NWM)], f32, tag="rstd")
                nw = n_win
                nc.vector.tensor_scalar_mul(negmu_t[:, :nw], sums_t[:, :nw], -1.0 / 128)
                nm2 = work.tile([128, max(NWJ, NWM)], f32, tag="nm2")
                nc.vector.tensor_mul(nm2[:, :nw], negmu_t[:, :nw], negmu_t[:, :nw])
                var_t = work.tile([128, max(NWJ, NWM)], f32, tag="var")
                nc.vector.tensor_scalar_mul(var_t[:, :nw], sumsq_t[:, :nw], 1.0 / 128)
                nc.vector.tensor_sub(var_t[:, :nw], var_t[:, :nw], nm2[:, :nw])
                std_t = work.tile([128, max(NWJ, NWM)], f32, tag="std")
                nc.scalar.activation(std_t[:, :nw], var_t[:, :nw], Act.Sqrt,
                                     bias=eps_t[:])
                nc.vector.reciprocal(rstd_t[:, :nw], std_t[:, :nw])

                for w in range(n_win):
                    nc.vector.tensor_scalar(
                        slab[:, w, :], slab[:, w, :],
                        negmu_t[:, w:w + 1], rstd_t[:, w:w + 1],
                        Alu.add, Alu.mult)
                    if ln_t is not None:
                        nc.vector.tensor_mul(slab[:, w, :], slab[:, w, :], ln_t[:, 0, :])
                        nc.vector.tensor_add(slab[:, w, :], slab[:, w, :], ln_t[:, 1, :])
                    nc.sync.dma_start(out_d[w * 128:(w + 1) * 128, :], slab[:, w, :])

            def whole():
                emit_pass(NWJ, cfg["p1_groups"], idx1_d, dst1_d, jobT, cfg["nj_pad"],
                          mach_tab, cfg["n_mach_tab"], cfg["n_mach_tab"],
                          wjt1_t, wjt2_t, bj_t, cfg["bias_j"], lnj_t, outj_d)
                emit_pass(NWM, cfg["p2_groups"], idx2_d, dst2_d, machT, cfg["nm_pad"],
                          job_tab, cfg["n_job_tab"], BUCKET,
                          wmt1_t, wmt2_t, bm_t, cfg["bias_m"], lnm_t, outm_d)

            if cfg.get("repeat", 1) == 1:
                whole()
            else:
                with tc.For_i(0, cfg["repeat"], 1):
                    whole()

    nc.compile()
    return nc


# ---------------------------------------------------------------- entry point

def _prepare(inputs, repeat=1):
    job_h = np.asarray(inputs["job_h"], np.float32)
    machine_h = np.asarray(inputs["machine_h"], np.float32)
    W_job_w = np.asarray(inputs["W_job_w"], np.float32)
    W_job_b = np.asarray(inputs["W_job_b"], np.float32)
    W_machine_w = np.asarray(inputs["W_machine_w"], np.float32)
    W_machine_b = np.asarray(inputs["W_machine_b"], np.float32)
    ln_j_scale = np.asarray(inputs["ln_j_scale"], np.float32)
    ln_j_bias = np.asarray(inputs["ln_j_bias"], np.float32)
    ln_m_scale = np.asarray(inputs["ln_m_scale"], np.float32)
    ln_m_bias = np.asarray(inputs["ln_m_bias"], np.float32)
    job_idx = np.asarray(inputs["job_idx"], np.int64)
    machine_idx = np.asarray(inputs["machine_idx"], np.int64)

    nj_pad = _pad128(JPC)
    nm_pad = _pad128(MPC)

    # pass 1: dest = jobs, sources = machines (single bucket)
    p1_groups, T1, src1, dst1 = _prep_pass(job_idx, machine_idx, JPC, 1, N_MACH)
    # pass 2: dest = machines, sources = jobs (int16 buckets)
    nb2 = (N_JOBS + BUCKET - 1) // BUCKET
    p2_groups, T2, src2, dst2 = _prep_pass(machine_idx, job_idx, MPC, nb2, BUCKET)

    idx1 = _wrap_idx(src1)
    dst1c = _wrap_dst(dst1)
    idx2 = _wrap_idx(src2)
    dst2c = _wrap_dst(dst2)

    mach_tab = machine_h.astype(bf16)
    job_tab = job_h.astype(bf16)

    jobT = np.zeros((N_CORES, 128, nj_pad), bf16)
    machT = np.zeros((N_CORES, 128, nm_pad), bf16)
    for c in range(N_CORES):
        jobT[c, :, :JPC] = job_h[c * JPC:(c + 1) * JPC].T.astype(bf16)
        machT[c, :, :MPC] = machine_h[c * MPC:(c + 1) * MPC].T.astype(bf16)

    cfg = {
        "nj_pad": nj_pad, "nm_pad": nm_pad,
        "n_mach_tab": N_MACH, "n_job_tab": N_JOBS,
        "T1": T1, "T2": T2,
        "p1_groups": p1_groups, "p2_groups": p2_groups,
        "bias_j": bool(np.any(W_job_b != 0)),
        "bias_m": bool(np.any(W_machine_b != 0)),
        "affine_j": not (np.all(ln_j_scale == 1) and np.all(ln_j_bias == 0)),
        "affine_m": not (np.all(ln_m_scale == 1) and np.all(ln_m_bias == 0)),
        "repeat": repeat,
    }

    iota_np = np.broadcast_to(np.arange(128, dtype=np.float32), (128, 128)).astype(bf16)
    ident_np = np.eye(128, dtype=np.float32).astype(bf16)

    lnj_np = np.stack([np.tile(ln_j_scale, (128, 1)), np.tile(ln_j_bias, (128, 1))], 1).astype(np.float32)
    lnm_np = np.stack([np.tile(ln_m_scale, (128, 1)), np.tile(ln_m_bias, (128, 1))], 1).astype(np.float32)

    in_maps = []
    for c in range(N_CORES):
        in_maps.append({
            "mach_tab": mach_tab, "job_tab": job_tab,
            "jobT": np.ascontiguousarray(jobT[c]),
            "machT": np.ascontiguousarray(machT[c]),
            "idx1": idx1[c], "dst1": dst1c[c],
            "idx2": idx2[c], "dst2": dst2c[c],
            "wjt1": np.ascontiguousarray(W_job_w[:, :D].T).astype(bf16),
            "wjt2": np.ascontiguousarray(W_job_w[:, D:].T).astype(bf16),
            "wmt1": np.ascontiguousarray(W_machine_w[:, :D].T).astype(bf16),
            "wmt2": np.ascontiguousarray(W_machine_w[:, D:].T).astype(bf16),
            "bj": W_job_b[None, :].astype(bf16),
            "bm": W_machine_b[None, :].astype(bf16),
            "iota": iota_np, "ident": ident_np,
            "lnj": lnj_np, "lnm": lnm_np,
        })
    return cfg, in_maps


def kernel(**inputs):
    from concourse.bass_utils import run_bass_kernel_spmd
    cfg, in_maps = _prepare(inputs)
    nc = _build(cfg)
    res = run_bass_kernel_spmd(nc, in_maps, core_ids=list(range(N_CORES)))
    new_job = np.concatenate(
        [res.results[c]["outj"][:JPC] for c in range(N_CORES)], axis=0)
    new_mach = np.concatenate(
        [res.results[c]["outm"][:MPC] for c in range(N_CORES)], axis=0)
    return (np.ascontiguousarray(new_job), np.ascontiguousarray(new_mach))


# revision 12
# speedup vs baseline: 1.2121x; 1.2121x over previous
"""Bass/Trainium2 kernel for nn_BipartiteSAGELayer (bipartite GraphSAGE layer).

Strategy (8 NeuronCores, SPMD, no collectives):
  - Shard DESTINATION nodes across cores: core c owns jobs [c*12500, (c+1)*12500)
    and machines [c*1250, (c+1)*1250).
  - Host prep ("sharding"): sort/bucket edges per (core, dest-window-of-128
    [, source-bucket-of-32768]) with padding so every core runs an identical
    static program; cast feature tables to bf16; pre-transpose per-core node
    features.
  - Device per dest-window: dma_gather source rows (bf16, 256B) from the DRAM
    table; build one-hot [edge, dest] tiles on DVE (is_equal vs an iota row);
    TensorE: agg[d,f] += onehot.T @ gathered, deg[d] += onehot.T @ ones;
    scale rows by 1/clip(deg,1) (ACT copy with per-partition scale), PE
    transpose -> aggT; linear new = [h | agg] @ W.T via hT/aggT stationaries;
    relu+LN (batched moment math) -> output rows.
"""

import numpy as np
import ml_dtypes

N_JOBS = 100000
N_MACH = 10000
N_EDGES = 1600000
D = 128
OUT = 128
LN_EPS = 1e-5
N_CORES = 8
JPC = N_JOBS // N_CORES       # 12500
MPC = N_MACH // N_CORES       # 1250
BUCKET = 32768                # int16-safe source bucket for the job table
GB = 64                       # max chunks (of 128 edges) per dma_gather block

bf16 = ml_dtypes.bfloat16


# ---------------------------------------------------------------- host prep

def _pad128(x):
    return (int(x) + 127) // 128 * 128


def _prep_pass(dst, src, n_dest_pc, n_buckets, bucket_size):
    """Shard edges by (core, dest-window [,src-bucket]) with a static layout.

    Returns (groups, T, src_l, dst_l):
      groups: per window, list of (bucket, n_chunks) with n_chunks >= 0;
              layout is window-major, bucket-sub-major, each group padded
              to a multiple of 128 slots. Identical across cores.
      T: total slots per core.
      src_l: [n_cores, T] int16 bucket-local source indices (pads = 0)
      dst_l: [n_cores, T] float32 window-local dest indices (pads = -1)
    """
    dst = dst.astype(np.int64)
    src = src.astype(np.int64)
    n_win = _pad128(n_dest_pc) // 128
    core = dst // n_dest_pc
    dloc = dst % n_dest_pc
    win = dloc // 128
    buck = src // bucket_size if n_buckets > 1 else np.zeros_like(src)
    nk = N_CORES * n_win * n_buckets
    key = (core * n_win + win) * n_buckets + buck
    cnt = np.bincount(key, minlength=nk).reshape(N_CORES, n_win, n_buckets)
    grp = cnt.max(axis=0)                      # [n_win, n_buckets]
    grp_pad = (grp + 127) // 128 * 128
    for w in range(n_win):                     # ensure >=1 chunk per window
        if grp_pad[w].sum() == 0:
            grp_pad[w, 0] = 128
    # flat offsets (same for every core)
    off = np.zeros((n_win, n_buckets), np.int64)
    t = 0
    groups = []
    for w in range(n_win):
        gl = []
        for b in range(n_buckets):
            off[w, b] = t
            t += grp_pad[w, b]
            if grp_pad[w, b]:
                gl.append((b, int(grp_pad[w, b]) // 128))
        groups.append(gl)
    T = int(t)

    # slot of each edge: off[win, buck] + rank within its (core,win,buck) group
    order = np.argsort(key, kind="stable")
    sk = key[order]
    first = np.zeros(nk, np.int64)
    first[1:] = np.cumsum(np.bincount(key, minlength=nk))[:-1]
    rank = np.arange(len(dst)) - first[sk]
    e_core = core[order]
    e_win = win[order]
    e_buck = buck[order]
    slot = off[e_win, e_buck] + rank

    src_l = np.zeros((N_CORES, T), np.int16)
    dst_l = np.full((N_CORES, T), -1.0, np.float32)
    src_l[e_core, slot] = (src[order] - e_buck * bucket_size).astype(np.int16)
    dst_l[e_core, slot] = (dloc[order] - e_win * 128).astype(np.float32)
    return groups, T, src_l, dst_l


def _wrap_idx(src_l):
    """[T] int16 -> [128, T//16] gather-index layout (16-wrapped, tiled x8)."""
    T = src_l.shape[-1]
    w16 = src_l.reshape(-1, T // 16, 16).transpose(0, 2, 1)  # [cores, 16, T/16]
    return np.ascontiguousarray(np.tile(w16, (1, 8, 1)))     # [cores, 128, T/16]


def _wrap_dst(dst_l):
    """[T] f32 -> [128, T//128] bf16 per-chunk dest columns."""
    T = dst_l.shape[-1]
    d = dst_l.reshape(-1, T // 128, 128).transpose(0, 2, 1)
    return np.ascontiguousarray(d).astype(bf16)


# ---------------------------------------------------------------- device build

def _build(cfg):
    import concourse.bacc as bacc
    import concourse.tile as tile
    import concourse.mybir as mybir
    from concourse import library_config

    f32 = mybir.dt.float32
    b16 = mybir.dt.bfloat16
    i16 = mybir.dt.int16
    Alu = mybir.AluOpType
    Act = mybir.ActivationFunctionType

    nc = bacc.Bacc("TRN2", target_bir_lowering=False, num_swdge_queues=4)
    gq = [0]  # round-robin gather queue counter

    mach_tab = nc.dram_tensor("mach_tab", [cfg["n_mach_tab"], D], b16, kind="ExternalInput")
    job_tab = nc.dram_tensor("job_tab", [cfg["n_job_tab"], D], b16, kind="ExternalInput")
    jobT = nc.dram_tensor("jobT", [128, cfg["nj_pad"]], b16, kind="ExternalInput")
    machT = nc.dram_tensor("machT", [128, cfg["nm_pad"]], b16, kind="ExternalInput")
    idx1_d = nc.dram_tensor("idx1", [128, cfg["T1"] // 16], i16, kind="ExternalInput")
    dst1_d = nc.dram_tensor("dst1", [128, cfg["T1"] // 128], b16, kind="ExternalInput")
    idx2_d = nc.dram_tensor("idx2", [128, cfg["T2"] // 16], i16, kind="ExternalInput")
    dst2_d = nc.dram_tensor("dst2", [128, cfg["T2"] // 128], b16, kind="ExternalInput")
    wjt1_d = nc.dram_tensor("wjt1", [128, 128], b16, kind="ExternalInput")
    wjt2_d = nc.dram_tensor("wjt2", [128, 128], b16, kind="ExternalInput")
    wmt1_d = nc.dram_tensor("wmt1", [128, 128], b16, kind="ExternalInput")
    wmt2_d = nc.dram_tensor("wmt2", [128, 128], b16, kind="ExternalInput")
    bj_d = nc.dram_tensor("bj", [1, 128], b16, kind="ExternalInput")
    bm_d = nc.dram_tensor("bm", [1, 128], b16, kind="ExternalInput")
    iota_d = nc.dram_tensor("iota", [128, 128], b16, kind="ExternalInput")
    ident_d = nc.dram_tensor("ident", [128, 128], b16, kind="ExternalInput")
    lnj_d = nc.dram_tensor("lnj", [128, 2, 128], f32, kind="ExternalInput")
    lnm_d = nc.dram_tensor("lnm", [128, 2, 128], f32, kind="ExternalInput")
    outj_d = nc.dram_tensor("outj", [cfg["nj_pad"], 128], f32, kind="ExternalOutput")
    outm_d = nc.dram_tensor("outm", [cfg["nm_pad"], 128], f32, kind="ExternalOutput")

    NWJ = cfg["nj_pad"] // 128
    NWM = cfg["nm_pad"] // 128

    with tile.TileContext(nc) as tc:
        with (
            tc.tile_pool(name="consts", bufs=1) as consts,
            tc.tile_pool(name="big", bufs=1) as big,
            tc.tile_pool(name="gpool", bufs=2) as gpool,
            tc.tile_pool(name="work", bufs=3) as work,
            tc.tile_pool(name="small", bufs=3) as small,
            tc.tile_pool(name="psum", bufs=2, space="PSUM") as psum,
        ):
            nc.gpsimd.load_library(library_config.mlp)

            iota_t = consts.tile([128, 128], b16)
            ident_t = consts.tile([128, 128], b16)
            onec_t = consts.tile([128, 1], b16)
            ones1_t = consts.tile([1, 128], b16)
            wjt1_t = consts.tile([128, 128], b16)
            wjt2_t = consts.tile([128, 128], b16)
            wmt1_t = consts.tile([128, 128], b16)
            wmt2_t = consts.tile([128, 128], b16)
            bj_t = consts.tile([1, 128], b16)
            bm_t = consts.tile([1, 128], b16)
            nc.sync.dma_start(iota_t[:], iota_d[:])
            nc.sync.dma_start(ident_t[:], ident_d[:])
            nc.sync.dma_start(wjt1_t[:], wjt1_d[:])
            nc.sync.dma_start(wjt2_t[:], wjt2_d[:])
            nc.sync.dma_start(wmt1_t[:], wmt1_d[:])
            nc.sync.dma_start(wmt2_t[:], wmt2_d[:])
            nc.sync.dma_start(bj_t[:], bj_d[:])
            nc.sync.dma_start(bm_t[:], bm_d[:])
            nc.vector.memset(onec_t[:], 1.0)
            nc.vector.memset(ones1_t[:], 1.0)
            eps_t = consts.tile([128, 1], f32)
            nc.vector.memset(eps_t[:], float(LN_EPS))
            lnj_t = lnm_t = None
            if cfg["affine_j"]:
                lnj_t = consts.tile([128, 2, 128], f32)
                nc.sync.dma_start(lnj_t[:], lnj_d[:])
            if cfg["affine_m"]:
                lnm_t = consts.tile([128, 2, 128], f32)
                nc.sync.dma_start(lnm_t[:], lnm_d[:])

            def emit_pass(n_win, groups, idx_d_, dst_d_, hT_d, hT_cols, table_d,
                          table_rows, bucket_size, WT1, WT2, bias_t, use_bias,
                          ln_t, out_d):
                idx_t = big.tile([128, max(cfg["T1"], cfg["T2"]) // 16], i16, tag="idx")
                dst_t = big.tile([128, max(cfg["T1"], cfg["T2"]) // 128], b16, tag="dst")
                hT_t = big.tile([128, max(cfg["nj_pad"], cfg["nm_pad"])], b16, tag="hT")
                Tn = idx_d_.shape[1]
                nc.sync.dma_start(idx_t[:, :Tn], idx_d_[:])
                nc.sync.dma_start(dst_t[:, :Tn // 8], dst_d_[:])
                nc.sync.dma_start(hT_t[:, :hT_cols], hT_d[:])

                slab = big.tile([128, max(NWJ, NWM), 128], f32, tag="slab")
                sums_t = big.tile([128, max(NWJ, NWM)], f32, tag="sums")
                sumsq_t = big.tile([128, max(NWJ, NWM)], f32, tag="sumsq")

                chunk_off = 0
                for w in range(n_win):
                    agg_p = psum.tile([128, 128], f32, tag="agg")
                    deg_p = psum.tile([128, 1], f32, tag="deg")
                    wch = sum(nch for _, nch in groups[w])
                    ci = 0
                    for (b, nch) in groups[w]:
                        lo = b * bucket_size
                        hi = min((b + 1) * bucket_size, table_rows)
                        tab_ap = table_d[lo:hi, :]
                        for blk in range(0, nch, GB):
                            nb = min(GB, nch - blk)
                            co = chunk_off + ci
                            g_t = gpool.tile([128, GB, 128], b16, tag="gath")
                            nc.gpsimd.dma_gather(
                                g_t[:, :nb, :], tab_ap,
                                idx_t[:, co * 8:(co + nb) * 8],
                                nb * 128, nb * 128, 128, single_packet=False,
                                queue_num=gq[0] % 4)
                            gq[0] += 1
                            oh_t = gpool.tile([128, GB, 128], b16, tag="oh")
                            nc.vector.tensor_tensor(
                                oh_t[:, :nb, :],
                                dst_t[:, co:co + nb].unsqueeze(2).broadcast_to([128, nb, 128]),
                                iota_t[:].unsqueeze(1).broadcast_to([128, nb, 128]),
                                Alu.is_equal)
                            for c in range(nb):
                                st = (ci == 0)
                                sp = (ci == wch - 1)
                                nc.tensor.matmul(agg_p[:], oh_t[:, c, :], g_t[:, c, :],
                                                 start=st, stop=sp)
                                nc.tensor.matmul(deg_p[:], oh_t[:, c, :], onec_t[:],
                                                 start=st, stop=sp)
                                ci += 1
                    chunk_off += wch

                    degc_t = small.tile([128, 1], f32, tag="degc")
                    nc.vector.tensor_scalar_max(degc_t[:], deg_p[:], 1.0)
                    recip_t = small.tile([128, 1], f32, tag="recip")
                    nc.vector.reciprocal(recip_t[:], degc_t[:])
                    mean_s = work.tile([128, 128], b16, tag="means")
                    nc.scalar.activation(mean_s[:], agg_p[:], Act.Copy,
                                         bias=0.0, scale=recip_t[:])
                    meanT_p = psum.tile([128, 128], b16, tag="mT")
                    nc.tensor.transpose(meanT_p[:], mean_s[:], ident_t[:])
                    aggT_s = work.tile([128, 128], b16, tag="aggT")
                    nc.scalar.activation(aggT_s[:], meanT_p[:], Act.Copy)

                    pre_p = psum.tile([128, 128], f32, tag="lin")
                    nc.tensor.matmul(pre_p[:], hT_t[:, w * 128:(w + 1) * 128], WT1[:],
                                     start=True, stop=False)
                    nc.tensor.matmul(pre_p[:], aggT_s[:], WT2[:],
                                     start=False, stop=not use_bias)
                    if use_bias:
                        nc.tensor.matmul(pre_p[:], ones1_t[:], bias_t[:],
                                         start=False, stop=True)
                    nc.scalar.activation(slab[:, w, :], pre_p[:], Act.Relu,
                                         accum_out=sums_t[:, w:w + 1])
                    sq_t = work.tile([128, 128], f32, tag="sqscr")
                    nc.scalar.activation(sq_t[:], slab[:, w, :], Act.Square,
                                         accum_out=sumsq_t[:, w:w + 1])

                # batched LN moments
                negmu_t = big.tile([128, max(NWJ, NWM)], f32, tag="negmu")
                rstd_t = big.tile([128, max(NWJ, NWM)], f32, tag="rstd")
                nw = n_win
                nc.vector.tensor_scalar_mul(negmu_t[:, :nw], sums_t[:, :nw], -1.0 / 128)
                nm2 = work.tile([128, max(NWJ, NWM)], f32, tag="nm2")
                nc.vector.tensor_mul(nm2[:, :nw], negmu_t[:, :nw], negmu_t[:, :nw])
                var_t = work.tile([128, max(NWJ, NWM)], f32, tag="var")
                nc.vector.tensor_scalar_mul(var_t[:, :nw], sumsq_t[:, :nw], 1.0 / 128)
                nc.vector.tensor_sub(var_t[:, :nw], var_t[:, :nw], nm2[:, :nw])
                std_t = work.tile([128, max(NWJ, NWM)], f32, tag="std")
                nc.scalar.activation(std_t[:, :nw], var_t[:, :nw], Act.Sqrt,
                                     bias=eps_t[:])
                nc.vector.reciprocal(rstd_t[:, :nw], std_t[:, :nw])

                for w in range(n_win):
                    nc.vector.tensor_scalar(
                        slab[:, w, :], slab[:, w, :],
                        negmu_t[:, w:w + 1], rstd_t[:, w:w + 1],
                        Alu.add, Alu.mult)
                    if ln_t is not None:
                        nc.vector.tensor_mul(slab[:, w, :], slab[:, w, :], ln_t[:, 0, :])
                        nc.vector.tensor_add(slab[:, w, :], slab[:, w, :], ln_t[:, 1, :])
                    nc.sync.dma_start(out_d[w * 128:(w + 1) * 128, :], slab[:, w, :])

            def whole():
                emit_pass(NWJ, cfg["p1_groups"], idx1_d, dst1_d, jobT, cfg["nj_pad"],
                          mach_tab, cfg["n_mach_tab"], cfg["n_mach_tab"],
                          wjt1_t, wjt2_t, bj_t, cfg["bias_j"], lnj_t, outj_d)
                emit_pass(NWM, cfg["p2_groups"], idx2_d, dst2_d, machT, cfg["nm_pad"],
                          job_tab, cfg["n_job_tab"], BUCKET,
                          wmt1_t, wmt2_t, bm_t, cfg["bias_m"], lnm_t, outm_d)

            if cfg.get("repeat", 1) == 1:
                whole()
            else:
                with tc.For_i(0, cfg["repeat"], 1):
                    whole()

    nc.compile()
    return nc


# ---------------------------------------------------------------- entry point

def _prepare(inputs, repeat=1):
    job_h = np.asarray(inputs["job_h"], np.float32)
    machine_h = np.asarray(inputs["machine_h"], np.float32)
    W_job_w = np.asarray(inputs["W_job_w"], np.float32)
    W_job_b = np.asarray(inputs["W_job_b"], np.float32)
    W_machine_w = np.asarray(inputs["W_machine_w"], np.float32)
    W_machine_b = np.asarray(inputs["W_machine_b"], np.float32)
    ln_j_scale = np.asarray(inputs["ln_j_scale"], np.float32)
    ln_j_bias = np.asarray(inputs["ln_j_bias"], np.float32)
    ln_m_scale = np.asarray(inputs["ln_m_scale"], np.float32)
    ln_m_bias = np.asarray(inputs["ln_m_bias"], np.float32)
    job_idx = np.asarray(inputs["job_idx"], np.int64)
    machine_idx = np.asarray(inputs["machine_idx"], np.int64)

    nj_pad = _pad128(JPC)
    nm_pad = _pad128(MPC)

    # pass 1: dest = jobs, sources = machines (single bucket)
    p1_groups, T1, src1, dst1 = _prep_pass(job_idx, machine_idx, JPC, 1, N_MACH)
    # pass 2: dest = machines, sources = jobs (int16 buckets)
    nb2 = (N_JOBS + BUCKET - 1) // BUCKET
    p2_groups, T2, src2, dst2 = _prep_pass(machine_idx, job_idx, MPC, nb2, BUCKET)

    idx1 = _wrap_idx(src1)
    dst1c = _wrap_dst(dst1)
    idx2 = _wrap_idx(src2)
    dst2c = _wrap_dst(dst2)

    mach_tab = machine_h.astype(bf16)
    job_tab = job_h.astype(bf16)

    jobT = np.zeros((N_CORES, 128, nj_pad), bf16)
    machT = np.zeros((N_CORES, 128, nm_pad), bf16)
    for c in range(N_CORES):
        jobT[c, :, :JPC] = job_h[c * JPC:(c + 1) * JPC].T.astype(bf16)
        machT[c, :, :MPC] = machine_h[c * MPC:(c + 1) * MPC].T.astype(bf16)

    cfg = {
        "nj_pad": nj_pad, "nm_pad": nm_pad,
        "n_mach_tab": N_MACH, "n_job_tab": N_JOBS,
        "T1": T1, "T2": T2,
        "p1_groups": p1_groups, "p2_groups": p2_groups,
        "bias_j": bool(np.any(W_job_b != 0)),
        "bias_m": bool(np.any(W_machine_b != 0)),
        "affine_j": not (np.all(ln_j_scale == 1) and np.all(ln_j_bias == 0)),
        "affine_m": not (np.all(ln_m_scale == 1) and np.all(ln_m_bias == 0)),
        "repeat": repeat,
    }

    iota_np = np.broadcast_to(np.arange(128, dtype=np.float32), (128, 128)).astype(bf16)
    ident_np = np.eye(128, dtype=np.float32).astype(bf16)

    lnj_np = np.stack([np.tile(ln_j_scale, (128, 1)), np.tile(ln_j_bias, (128, 1))], 1).astype(np.float32)
    lnm_np = np.stack([np.tile(ln_m_scale, (128, 1)), np.tile(ln_m_bias, (128, 1))], 1).astype(np.float32)

    in_maps = []
    for c in range(N_CORES):
        in_maps.append({
            "mach_tab": mach_tab, "job_tab": job_tab,
            "jobT": np.ascontiguousarray(jobT[c]),
            "machT": np.ascontiguousarray(machT[c]),
            "idx1": idx1[c], "dst1": dst1c[c],
            "idx2": idx2[c], "dst2": dst2c[c],
            "wjt1": np.ascontiguousarray(W_job_w[:, :D].T).astype(bf16),
            "wjt2": np.ascontiguousarray(W_job_w[:, D:].T).astype(bf16),
            "wmt1": np.ascontiguousarray(W_machine_w[:, :D].T).astype(bf16),
            "wmt2": np.ascontiguousarray(W_machine_w[:, D:].T).astype(bf16),
            "bj": W_job_b[None, :].astype(bf16),
            "bm": W_machine_b[None, :].astype(bf16),
            "iota": iota_np, "ident": ident_np,
            "lnj": lnj_np, "lnm": lnm_np,
        })
    return cfg, in_maps


def kernel(**inputs):
    from concourse.bass_utils import run_bass_kernel_spmd
    cfg, in_maps = _prepare(inputs)
    nc = _build(cfg)
    res = run_bass_kernel_spmd(nc, in_maps, core_ids=list(range(N_CORES)))
    new_job = np.concatenate(
        [res.results[c]["outj"][:JPC] for c in range(N_CORES)], axis=0)
    new_mach = np.concatenate(
        [res.results[c]["outm"][:MPC] for c in range(N_CORES)], axis=0)
    return (np.ascontiguousarray(new_job), np.ascontiguousarray(new_mach))
